# revision 13
# baseline (speedup 1.0000x reference)
"""DenseGATConv-style GNN message passing kernel for Trainium2 (Bass/Tile).

Math (per graph b):
    e      = w_edge[edge_attr[b]]            # [N, N] gather from 4-entry table
    adj_w  = adj[b] * e                      # weighted adjacency
    agg    = adj_w @ x[b]                    # [N, C]
    out[b] = agg @ W_rel + b_rel + x[b] @ W_root

Key tricks:
  * The 4-entry gather w_edge[a], a in {0,1,2,3}, equals the cubic polynomial
    through the 4 points, evaluated in factored form
        p(a)/c3 = (a - r) * ((a + h)^2 + v2)
    computed with one ScalarE Square activation + two fused
    scalar_tensor_tensor ops; c3 is folded into W_rel on the host.
  * b_rel is folded into the W_rel matmul as a 65th contraction row against a
    constant ones-row appended to agg^T.
  * The aggregation runs in transposed layout (out^T = Wrel^T@aggT + ...),
    with adj_w transposed on the PE in 128x128 blocks, 8 blocks batched per
    PSUM->SBUF copy.  The whole output stage runs per half-graph (512 rows)
    so the tail latency overlaps the next half's compute.

Sharding: data-parallel over batch B=16 across 8 cores (2 graphs/core);
weights replicated.
"""

import sys
from contextlib import ExitStack

sys.path.insert(0, "/opt/trn_rl_repo")

import numpy as np

_B, _N, _C = 16, 1024, 64
_NCORES = 8
_G = _B // _NCORES  # graphs per core
_P = 128
_NT = _N // _P  # 128-row tiles per graph

# Module-level knobs (test.py may flip these before calling kernel()).
TRACE = False
SQUARE_ENGINE = "act_sq"  # "act_sq" (ScalarE Square) | "dve_stt" (VectorE)
# "float32": exact, PE-bound (~4 cyc/row).  "float32r": TF32-class matmul
# precision (~1.5e-4 rel) but ~2x faster PE for transposes + aggregation.
MM_DTYPE = "float32r"
FINAL_DTYPE = "float32"  # dtype of the small output-transform matmuls
LAST_RESULTS = None  # BassKernelResults of the most recent run (for test.py)

_BUILD_CACHE = {}


def _poly_coeffs(w_edge):
    """Cubic through (k, w_edge[k]) for k=0..3, float64. Returns c0..c3."""
    w = np.asarray(w_edge, dtype=np.float64).reshape(4)
    V = np.vander(np.arange(4.0), 4, increasing=True)
    c = np.linalg.solve(V, w)
    return c  # [c0, c1, c2, c3]


def _chain_params(w_edge):
    """Pick the elementwise chain and host-folded scale from w_edge values.

    Returns (mode, params, lead) where `lead` multiplies W_rel on the host and
    the device computes adj_w/lead.
    """
    c0, c1, c2, c3 = _poly_coeffs(w_edge)
    scale = max(np.max(np.abs(np.asarray(w_edge, dtype=np.float64))), 1e-30)
    tol = 1e-7 * scale
    if abs(c3) > tol:
        # monic cubic a^3 + A a^2 + B a + C = (a - r)(a^2 + p a + q)
        A, Bc, Cc = c2 / c3, c1 / c3, c0 / c3
        roots = np.roots([1.0, A, Bc, Cc])
        r = float(np.real(roots[np.argmin(np.abs(np.imag(roots)))]))
        p = A + r
        q = Bc + p * r
        return "cubic", dict(r=r, p=p, q=q, h=p / 2.0, v2=q - p * p / 4.0), c3
    if abs(c2) > tol:
        p2, q2 = c1 / c2, c0 / c2
        return "quad", dict(p=p2, q=q2, h=p2 / 2.0, v2=q2 - p2 * p2 / 4.0), c2
    if abs(c1) > tol:
        return "linear", dict(r=-c0 / c1), c1
    return "const", dict(), c0


def _emit_square(nc, OP, AF, s_out, ea_ap, params, square_engine, pools):
    """s_out <- quadratic-part tensor; returns the constant to add to it."""
    if square_engine == "act_sq":
        nc.scalar.activation(
            s_out, ea_ap, AF.Square, bias=pools["hbias_sb"][:, 0:1], scale=1.0
        )
        return float(params["v2"])
    nc.vector.scalar_tensor_tensor(
        s_out, ea_ap, float(-params["p"]), ea_ap, OP.subtract, OP.mult
    )
    return float(params["q"])


def _emit_elementwise(nc, OP, AF, pools, ea_t, adj_t, mode, params, square_engine):
    """Emit adj_w/lead for one [128, N] tile slice pair; returns the aw tile."""
    sp, qtp, awp = pools["sp"], pools["qtp"], pools["awp"]
    mmdt = pools["mmdt"]
    f32 = pools["f32"]
    if mode == "cubic":
        qt_t = qtp.tile([_P, _N], f32)
        nc.vector.scalar_tensor_tensor(
            qt_t[:], ea_t, float(params["r"]), adj_t, OP.subtract, OP.mult
        )
        s_t = sp.tile([_P, _N], f32)
        k_add = _emit_square(nc, OP, AF, s_t[:], ea_t, params, square_engine, pools)
        aw_t = awp.tile([_P, _N], mmdt)
        nc.vector.scalar_tensor_tensor(
            aw_t[:], s_t[:], k_add, qt_t[:], OP.add, OP.mult
        )
        return aw_t
    if mode == "quad":
        s_t = sp.tile([_P, _N], f32)
        k_add = _emit_square(nc, OP, AF, s_t[:], ea_t, params, square_engine, pools)
        aw_t = awp.tile([_P, _N], mmdt)
        nc.vector.scalar_tensor_tensor(
            aw_t[:], s_t[:], k_add, adj_t, OP.add, OP.mult
        )
        return aw_t
    if mode == "linear":
        aw_t = awp.tile([_P, _N], mmdt)
        nc.vector.scalar_tensor_tensor(
            aw_t[:], ea_t, float(params["r"]), adj_t, OP.subtract, OP.mult
        )
        return aw_t
    aw_t = awp.tile([_P, _N], mmdt)
    nc.vector.tensor_copy(aw_t[:], adj_t)
    return aw_t


def _emit_half(nc, pools, g, half, dram, xs, xs_mm, xT, mode, params, square_engine):
    from concourse import mybir

    OP = mybir.AluOpType
    AF = mybir.ActivationFunctionType
    f32 = pools["f32"]
    mmdt = pools["mmdt"]
    fdt = pools["fdt"]
    adj_d, ea_d, out_d = dram["adj"], dram["ea"], dram["out"]
    ident = pools["ident"]
    ident_m = pools["ident_m"]
    H = 512

    # x^T columns for this half (root-term operand)
    p_xT = pools["ps_xt"].tile([_C, H], f32, tag="ps_xt")
    for k in range(4):
        jt = 4 * half + k
        nc.tensor.transpose(
            p_xT[:, k * _P : (k + 1) * _P],
            xs[:, jt * _C : (jt + 1) * _C],
            ident[:],
        )
    nc.scalar.copy(out=xT[:, half * H : (half + 1) * H], in_=p_xT[:])

    # DMA + elementwise for the half's 4 row-tiles (2 chunks of 2 tiles)
    aw_list = []
    for pair in range(2):
        base = 4 * half + 2 * pair
        ea_t = pools["eap"].tile([_P, 2 * _N], pools["i32"])
        nc.gpsimd.dma_start(
            out=ea_t[:].rearrange("p (q j) -> p q j", q=2),
            in_=ea_d[g, base * _P : (base + 2) * _P, :].rearrange(
                "(q p) j -> p q j", p=_P
            ),
        )
        adj_t = pools["adjp"].tile([_P, 2 * _N], f32)
        nc.gpsimd.dma_start(
            out=adj_t[:].rearrange("p (q j) -> p q j", q=2),
            in_=adj_d[g, base * _P : (base + 2) * _P, :].rearrange(
                "(q p) j -> p q j", p=_P
            ),
        )
        for q in range(2):
            aw_list.append(
                _emit_elementwise(
                    nc, OP, AF, pools,
                    ea_t[:, q * _N : (q + 1) * _N],
                    adj_t[:, q * _N : (q + 1) * _N],
                    mode, params, square_engine,
                )
            )

    # transpose adj_w blocks (jt-pair batched) + accumulate agg^T over j
    p_aggT = pools["ps_agg"].tile([_C, H], f32, tag="ps_agg")
    for jtp in range(4):
        p_tp = pools["ps_tp"].tile([_P, 2 * H], mmdt, tag="ps_tp")
        for sub in range(2):
            jt = 2 * jtp + sub
            for k in range(4):
                nc.tensor.transpose(
                    p_tp[:, sub * H + k * _P : sub * H + (k + 1) * _P],
                    aw_list[k][:, jt * _P : (jt + 1) * _P],
                    ident_m[:],
                )
        awT = pools["awTp"].tile([_P, 2 * H], mmdt)
        nc.scalar.copy(out=awT[:], in_=p_tp[:])
        for sub in range(2):
            jt = 2 * jtp + sub
            nc.tensor.matmul(
                p_aggT[:],
                lhsT=xs_mm[:, jt * _C : (jt + 1) * _C],
                rhs=awT[:, sub * H : (sub + 1) * H],
                start=(jt == 0),
                stop=(jt == _NT - 1),
            )

    aggT = pools["aggTp"].tile([_C + 1, H], fdt)
    nc.vector.memset(aggT[_C : _C + 1, :], 1.0)
    nc.scalar.copy(out=aggT[:_C, :], in_=p_aggT[:])

    # out^T[c', i-half] = [W_rel; b_rel]^T @ [aggT; 1] + W_root^T @ xT
    p_out = pools["ps_out"].tile([_C, H], f32, tag="ps_out")
    nc.tensor.matmul(
        p_out[:], lhsT=pools["wrel_sb"][:], rhs=aggT[:], start=True, stop=False
    )
    nc.tensor.matmul(
        p_out[:], lhsT=pools["wroot_sb"][:],
        rhs=xT[:, half * H : (half + 1) * H] if fdt is f32
        else pools["xT_mm"][:, half * H : (half + 1) * H],
        start=False, stop=True,
    )
    outT = pools["outTp"].tile([_C, H], f32)
    nc.scalar.copy(out=outT[:], in_=p_out[:])

    # back to natural [i, c] layout and store
    p_on = pools["ps_out"].tile([_P, 4 * _C], f32, tag="ps_out")
    for k in range(4):
        nc.tensor.transpose(
            p_on[:, k * _C : (k + 1) * _C],
            outT[:, k * _P : (k + 1) * _P],
            ident[:_C, :_C],
        )
    out_sb = pools["outp"].tile([_P, 4 * _C], f32)
    nc.scalar.copy(out=out_sb[:], in_=p_on[:])
    nc.sync.dma_start(
        out=out_d[g, half * H : (half + 1) * H, :].rearrange(
            "(t p) c -> p t c", p=_P
        ),
        in_=out_sb[:].rearrange("p (t c) -> p t c", t=4),
    )


def _emit_graph(nc, tc, pools, g, dram, mode, params, square_engine):
    f32 = pools["f32"]
    mmdt = pools["mmdt"]
    x_d = dram["x"]

    # x in aggregation layout: xs[p, t*C+c] = x[t*128+p, c]
    xs = pools["xsp"].tile([_P, _NT * _C], f32)
    nc.sync.dma_start(
        out=xs[:].rearrange("p (t c) -> p t c", t=_NT),
        in_=x_d[g, :, :].rearrange("(t p) c -> p t c", p=_P),
    )
    if mmdt is f32:
        xs_mm = xs
    else:
        xs_mm = pools["xsp"].tile([_P, _NT * _C], mmdt, tag="xs_mm")
        nc.vector.tensor_copy(xs_mm[:], xs[:])
    xT = pools["xTp"].tile([_C, _N], f32)

    for half in range(2):
        _emit_half(
            nc, pools, g, half, dram, xs, xs_mm, xT, mode, params, square_engine
        )


def _build_module(mode, params, square_engine, mm_dtype, final_dtype):
    import concourse.bass as bass  # noqa: F401
    from concourse import bacc, mybir
    from concourse.tile import TileContext

    f32 = mybir.dt.float32
    i32 = mybir.dt.int32
    mmdt = getattr(mybir.dt, mm_dtype)
    fdt = getattr(mybir.dt, final_dtype)
    assert fdt is f32, "FINAL_DTYPE other than float32 not wired up"

    nc = bacc.Bacc(
        "TRN2", target_bir_lowering=False, debug=False, num_devices=_NCORES
    )

    dram = {
        "x": nc.dram_tensor("x", [_G, _N, _C], f32, kind="ExternalInput"),
        "adj": nc.dram_tensor("adj", [_G, _N, _N], f32, kind="ExternalInput"),
        "ea": nc.dram_tensor("ea", [_G, _N, _N], i32, kind="ExternalInput"),
        "wrel": nc.dram_tensor("wrel", [_C + 1, _C], f32, kind="ExternalInput"),
        "wroot": nc.dram_tensor("wroot", [_C, _C], f32, kind="ExternalInput"),
        "ident": nc.dram_tensor("ident", [_P, _P], f32, kind="ExternalInput"),
        "out": nc.dram_tensor("out", [_G, _N, _C], f32, kind="ExternalOutput"),
    }

    pool_specs = [
        ("consts", 1, None),
        ("adjp", 4, None),
        ("eap", 4, None),
        ("sp", 3, None),
        ("qtp", 3, None),
        ("awp", 6, None),
        ("awTp", 3, None),
        ("xsp", 2, None),
        ("xTp", 2, None),
        ("aggTp", 2, None),
        ("outTp", 2, None),
        ("outp", 2, None),
        ("ps_tp", 2, "PSUM"),
        ("ps_agg", 2, "PSUM"),
        ("ps_xt", 1, "PSUM"),
        ("ps_out", 1, "PSUM"),
    ]

    with TileContext(nc) as tc, ExitStack() as ctx:
        pools = {"f32": f32, "i32": i32, "mmdt": mmdt, "fdt": fdt}
        for name, bufs, space in pool_specs:
            kw = {"space": space} if space else {}
            pools[name] = ctx.enter_context(tc.tile_pool(name=name, bufs=bufs, **kw))

        ident = pools["consts"].tile([_P, _P], f32, tag="ident")
        nc.sync.dma_start(out=ident[:], in_=dram["ident"][:, :])
        pools["ident"] = ident
        if mm_dtype == "float32":
            pools["ident_m"] = ident
        else:
            ident_m = pools["consts"].tile([_P, _P], mmdt, tag="ident_m")
            nc.vector.tensor_copy(ident_m[:], ident[:])
            pools["ident_m"] = ident_m
        for wname, shape in (("wrel", [_C + 1, _C]), ("wroot", [_C, _C])):
            t = pools["consts"].tile(shape, f32, tag=wname)
            nc.sync.dma_start(out=t[:], in_=dram[wname][:, :])
            pools[wname + "_sb"] = t

        if square_engine == "act_sq" and mode in ("cubic", "quad"):
            hb = pools["consts"].tile([_P, 1], f32, tag="hb")
            nc.vector.memset(hb[:], float(params["h"]))
            pools["hbias_sb"] = hb

        for g in range(_G):
            _emit_graph(nc, tc, pools, g, dram, mode, params, square_engine)

    nc.finalize()
    return nc


def _get_module(w_edge, square_engine, mm_dtype, final_dtype="float32"):
    mode, params, lead = _chain_params(w_edge)
    key = (
        mode,
        tuple(sorted((k, round(v, 15)) for k, v in params.items())),
        square_engine,
        mm_dtype,
        final_dtype,
    )
    if key not in _BUILD_CACHE:
        _BUILD_CACHE[key] = _build_module(
            mode, params, square_engine, mm_dtype, final_dtype
        )
    return _BUILD_CACHE[key], lead


def _prep_inputs(x, adj, edge_attr, W_rel, b_rel, W_root, w_edge):
    x = np.ascontiguousarray(np.asarray(x, dtype=np.float32))
    adj = np.ascontiguousarray(np.asarray(adj, dtype=np.float32))
    ea = np.ascontiguousarray(np.asarray(edge_attr, dtype=np.int32).reshape(_B, _N, _N))
    W_rel = np.asarray(W_rel, dtype=np.float64)
    W_root = np.ascontiguousarray(np.asarray(W_root, dtype=np.float32))
    b_rel = np.asarray(b_rel, dtype=np.float32).reshape(1, _C)
    w_edge = np.asarray(w_edge)
    return x, adj, ea, W_rel, b_rel, W_root, w_edge


def kernel(x, adj, edge_attr, W_rel, b_rel, W_root, w_edge):
    global LAST_RESULTS
    from concourse.bass_utils import run_bass_kernel_spmd

    x, adj, ea, W_rel, b_rel, W_root, w_edge = _prep_inputs(
        x, adj, edge_attr, W_rel, b_rel, W_root, w_edge
    )
    nc, lead = _get_module(w_edge, SQUARE_ENGINE, MM_DTYPE, FINAL_DTYPE)
    wrel_eff = np.ascontiguousarray(
        np.concatenate([lead * W_rel, b_rel.astype(np.float64)], axis=0).astype(
            np.float32
        )
    )
    ident = np.eye(_P, dtype=np.float32)

    in_maps = []
    for c in range(_NCORES):
        sl = slice(c * _G, (c + 1) * _G)
        in_maps.append(
            {
                "x": x[sl],
                "adj": adj[sl],
                "ea": ea[sl],
                "wrel": wrel_eff,
                "wroot": W_root,
                "ident": ident,
            }
        )

    res = run_bass_kernel_spmd(nc, in_maps, list(range(_NCORES)), trace=TRACE)
    LAST_RESULTS = res
    out = np.concatenate([res.results[c]["out"] for c in range(_NCORES)], axis=0)
    return out


# revision 14
# speedup vs baseline: 1.1392x; 1.1392x over previous
"""DenseGATConv-style GNN message passing kernel for Trainium2 (Bass/Tile).

Math (per graph b):
    e      = w_edge[edge_attr[b]]            # [N, N] gather from 4-entry table
    adj_w  = adj[b] * e                      # weighted adjacency
    agg    = adj_w @ x[b]                    # [N, C]
    out[b] = agg @ W_rel + b_rel + x[b] @ W_root

Key tricks:
  * The 4-entry gather w_edge[a], a in {0,1,2,3}, equals the cubic polynomial
    through the 4 points, evaluated in factored form
        p(a)/c3 = (a - r) * ((a + h)^2 + v2)
    computed with one ScalarE Square activation + two fused
    scalar_tensor_tensor ops; c3 is folded into W_rel on the host.
  * b_rel is folded into the W_rel matmul as a 65th contraction row against a
    constant ones-row appended to agg^T.
  * The aggregation runs in transposed layout (out^T = Wrel^T@aggT + ...),
    with adj_w transposed on the PE in 128x128 blocks, 8 blocks batched per
    PSUM->SBUF copy.  The whole output stage runs per half-graph (512 rows)
    so the tail latency overlaps the next half's compute.

Sharding: data-parallel over batch B=16 across 8 cores (2 graphs/core);
weights replicated.
"""

import sys
from contextlib import ExitStack

sys.path.insert(0, "/opt/trn_rl_repo")

import numpy as np

_B, _N, _C = 16, 1024, 64
_NCORES = 8
_G = _B // _NCORES  # graphs per core
_P = 128
_NT = _N // _P  # 128-row tiles per graph

# Module-level knobs (test.py may flip these before calling kernel()).
TRACE = False
SQUARE_ENGINE = "act_sq"  # "act_sq" (ScalarE Square) | "dve_stt" (VectorE)
# "float32": exact, PE-bound (~4 cyc/row).  "float32r": TF32-class matmul
# precision (~1.5e-4 rel) but ~2x faster PE for transposes + aggregation.
MM_DTYPE = "float32r"
FINAL_DTYPE = "float32"  # dtype of the small output-transform matmuls
LAST_RESULTS = None  # BassKernelResults of the most recent run (for test.py)

_BUILD_CACHE = {}


def _poly_coeffs(w_edge):
    """Cubic through (k, w_edge[k]) for k=0..3, float64. Returns c0..c3."""
    w = np.asarray(w_edge, dtype=np.float64).reshape(4)
    V = np.vander(np.arange(4.0), 4, increasing=True)
    c = np.linalg.solve(V, w)
    return c  # [c0, c1, c2, c3]


def _chain_params(w_edge):
    """Pick the elementwise chain and host-folded scale from w_edge values.

    Returns (mode, params, lead) where `lead` multiplies W_rel on the host and
    the device computes adj_w/lead.
    """
    c0, c1, c2, c3 = _poly_coeffs(w_edge)
    scale = max(np.max(np.abs(np.asarray(w_edge, dtype=np.float64))), 1e-30)
    tol = 1e-7 * scale
    if abs(c3) > tol:
        # monic cubic a^3 + A a^2 + B a + C = (a - r)(a^2 + p a + q)
        A, Bc, Cc = c2 / c3, c1 / c3, c0 / c3
        roots = np.roots([1.0, A, Bc, Cc])
        r = float(np.real(roots[np.argmin(np.abs(np.imag(roots)))]))
        p = A + r
        q = Bc + p * r
        return "cubic", dict(r=r, p=p, q=q, h=p / 2.0, v2=q - p * p / 4.0), c3
    if abs(c2) > tol:
        p2, q2 = c1 / c2, c0 / c2
        return "quad", dict(p=p2, q=q2, h=p2 / 2.0, v2=q2 - p2 * p2 / 4.0), c2
    if abs(c1) > tol:
        return "linear", dict(r=-c0 / c1), c1
    return "const", dict(), c0


def _emit_square(nc, OP, AF, s_out, ea_ap, params, square_engine, pools):
    """s_out <- quadratic-part tensor; returns the constant to add to it."""
    if square_engine == "act_sq":
        nc.scalar.activation(
            s_out, ea_ap, AF.Square, bias=pools["hbias_sb"][:, 0:1], scale=1.0
        )
        return float(params["v2"])
    nc.vector.scalar_tensor_tensor(
        s_out, ea_ap, float(-params["p"]), ea_ap, OP.subtract, OP.mult
    )
    return float(params["q"])


def _emit_elementwise(nc, OP, AF, pools, ea_t, adj_t, mode, params, square_engine):
    """Emit adj_w/lead for one [128, N] tile slice pair; returns the aw tile."""
    sp, qtp, awp = pools["sp"], pools["qtp"], pools["awp"]
    mmdt = pools["mmdt"]
    f32 = pools["f32"]
    if mode == "cubic":
        qt_t = qtp.tile([_P, _N], f32)
        nc.vector.scalar_tensor_tensor(
            qt_t[:], ea_t, float(params["r"]), adj_t, OP.subtract, OP.mult
        )
        s_t = sp.tile([_P, _N], f32)
        k_add = _emit_square(nc, OP, AF, s_t[:], ea_t, params, square_engine, pools)
        aw_t = awp.tile([_P, _N], mmdt)
        nc.vector.scalar_tensor_tensor(
            aw_t[:], s_t[:], k_add, qt_t[:], OP.add, OP.mult
        )
        return aw_t
    if mode == "quad":
        s_t = sp.tile([_P, _N], f32)
        k_add = _emit_square(nc, OP, AF, s_t[:], ea_t, params, square_engine, pools)
        aw_t = awp.tile([_P, _N], mmdt)
        nc.vector.scalar_tensor_tensor(
            aw_t[:], s_t[:], k_add, adj_t, OP.add, OP.mult
        )
        return aw_t
    if mode == "linear":
        aw_t = awp.tile([_P, _N], mmdt)
        nc.vector.scalar_tensor_tensor(
            aw_t[:], ea_t, float(params["r"]), adj_t, OP.subtract, OP.mult
        )
        return aw_t
    aw_t = awp.tile([_P, _N], mmdt)
    nc.vector.tensor_copy(aw_t[:], adj_t)
    return aw_t


def _emit_half(nc, pools, g, half, dram, xs, xs_mm, xT, mode, params, square_engine):
    from concourse import mybir

    OP = mybir.AluOpType
    AF = mybir.ActivationFunctionType
    f32 = pools["f32"]
    mmdt = pools["mmdt"]
    fdt = pools["fdt"]
    adj_d, ea_d, out_d = dram["adj"], dram["ea"], dram["out"]
    ident = pools["ident"]
    ident_m = pools["ident_m"]
    H = 512

    # x^T columns for this half (root-term operand)
    p_xT = pools["ps_xt"].tile([_C, H], f32, tag="ps_xt")
    for k in range(4):
        jt = 4 * half + k
        nc.tensor.transpose(
            p_xT[:, k * _P : (k + 1) * _P],
            xs[:, jt * _C : (jt + 1) * _C],
            ident[:],
        )
    nc.scalar.copy(out=xT[:, half * H : (half + 1) * H], in_=p_xT[:])

    # DMA + elementwise for the half's 4 row-tiles (one 2 MiB chunk each)
    aw_list = []
    base = 4 * half
    ea_t = pools["eap"].tile([_P, 4 * _N], pools["i32"])
    nc.sync.dma_start(
        out=ea_t[:].rearrange("p (q j) -> p q j", q=4),
        in_=ea_d[g, base * _P : (base + 4) * _P, :].rearrange(
            "(q p) j -> p q j", p=_P
        ),
    )
    adj_t = pools["adjp"].tile([_P, 4 * _N], f32)
    nc.sync.dma_start(
        out=adj_t[:].rearrange("p (q j) -> p q j", q=4),
        in_=adj_d[g, base * _P : (base + 4) * _P, :].rearrange(
            "(q p) j -> p q j", p=_P
        ),
    )
    for q in range(4):
        aw_list.append(
            _emit_elementwise(
                nc, OP, AF, pools,
                ea_t[:, q * _N : (q + 1) * _N],
                adj_t[:, q * _N : (q + 1) * _N],
                mode, params, square_engine,
            )
        )

    # transpose adj_w blocks (jt-pair batched) + accumulate agg^T over j
    p_aggT = pools["ps_agg"].tile([_C, H], f32, tag="ps_agg")
    for jtp in range(4):
        p_tp = pools["ps_tp"].tile([_P, 2 * H], mmdt, tag="ps_tp")
        for sub in range(2):
            jt = 2 * jtp + sub
            for k in range(4):
                nc.tensor.transpose(
                    p_tp[:, sub * H + k * _P : sub * H + (k + 1) * _P],
                    aw_list[k][:, jt * _P : (jt + 1) * _P],
                    ident_m[:],
                )
        awT = pools["awTp"].tile([_P, 2 * H], mmdt)
        nc.scalar.copy(out=awT[:], in_=p_tp[:])
        for sub in range(2):
            jt = 2 * jtp + sub
            nc.tensor.matmul(
                p_aggT[:],
                lhsT=xs_mm[:, jt * _C : (jt + 1) * _C],
                rhs=awT[:, sub * H : (sub + 1) * H],
                start=(jt == 0),
                stop=(jt == _NT - 1),
            )

    aggT = pools["aggTp"].tile([_C + 1, H], fdt)
    nc.vector.memset(aggT[_C : _C + 1, :], 1.0)
    nc.scalar.copy(out=aggT[:_C, :], in_=p_aggT[:])

    # out^T[c', i-half] = [W_rel; b_rel]^T @ [aggT; 1] + W_root^T @ xT
    p_out = pools["ps_out"].tile([_C, H], f32, tag="ps_out")
    nc.tensor.matmul(
        p_out[:], lhsT=pools["wrel_sb"][:], rhs=aggT[:], start=True, stop=False
    )
    nc.tensor.matmul(
        p_out[:], lhsT=pools["wroot_sb"][:],
        rhs=xT[:, half * H : (half + 1) * H] if fdt is f32
        else pools["xT_mm"][:, half * H : (half + 1) * H],
        start=False, stop=True,
    )
    outT = pools["outTp"].tile([_C, H], f32)
    nc.scalar.copy(out=outT[:], in_=p_out[:])

    # back to natural [i, c] layout and store
    p_on = pools["ps_out"].tile([_P, 4 * _C], f32, tag="ps_out")
    for k in range(4):
        nc.tensor.transpose(
            p_on[:, k * _C : (k + 1) * _C],
            outT[:, k * _P : (k + 1) * _P],
            ident[:_C, :_C],
        )
    out_sb = pools["outp"].tile([_P, 4 * _C], f32)
    nc.scalar.copy(out=out_sb[:], in_=p_on[:])
    nc.sync.dma_start(
        out=out_d[g, half * H : (half + 1) * H, :].rearrange(
            "(t p) c -> p t c", p=_P
        ),
        in_=out_sb[:].rearrange("p (t c) -> p t c", t=4),
    )


def _emit_graph(nc, tc, pools, g, dram, mode, params, square_engine):
    f32 = pools["f32"]
    mmdt = pools["mmdt"]
    x_d = dram["x"]

    # x in aggregation layout: xs[p, t*C+c] = x[t*128+p, c]
    xs = pools["xsp"].tile([_P, _NT * _C], f32)
    nc.sync.dma_start(
        out=xs[:].rearrange("p (t c) -> p t c", t=_NT),
        in_=x_d[g, :, :].rearrange("(t p) c -> p t c", p=_P),
    )
    if mmdt is f32:
        xs_mm = xs
    else:
        xs_mm = pools["xsp"].tile([_P, _NT * _C], mmdt, tag="xs_mm")
        nc.vector.tensor_copy(xs_mm[:], xs[:])
    xT = pools["xTp"].tile([_C, _N], f32)

    for half in range(2):
        _emit_half(
            nc, pools, g, half, dram, xs, xs_mm, xT, mode, params, square_engine
        )


def _build_module(mode, params, square_engine, mm_dtype, final_dtype):
    import concourse.bass as bass  # noqa: F401
    from concourse import bacc, mybir
    from concourse.tile import TileContext

    f32 = mybir.dt.float32
    i32 = mybir.dt.int32
    mmdt = getattr(mybir.dt, mm_dtype)
    fdt = getattr(mybir.dt, final_dtype)
    assert fdt is f32, "FINAL_DTYPE other than float32 not wired up"

    nc = bacc.Bacc(
        "TRN2", target_bir_lowering=False, debug=False, num_devices=_NCORES
    )

    dram = {
        "x": nc.dram_tensor("x", [_G, _N, _C], f32, kind="ExternalInput"),
        "adj": nc.dram_tensor("adj", [_G, _N, _N], f32, kind="ExternalInput"),
        "ea": nc.dram_tensor("ea", [_G, _N, _N], i32, kind="ExternalInput"),
        "wrel": nc.dram_tensor("wrel", [_C + 1, _C], f32, kind="ExternalInput"),
        "wroot": nc.dram_tensor("wroot", [_C, _C], f32, kind="ExternalInput"),
        "ident": nc.dram_tensor("ident", [_P, _P], f32, kind="ExternalInput"),
        "out": nc.dram_tensor("out", [_G, _N, _C], f32, kind="ExternalOutput"),
    }

    pool_specs = [
        ("consts", 1, None),
        ("adjp", 2, None),
        ("eap", 2, None),
        ("sp", 3, None),
        ("qtp", 3, None),
        ("awp", 6, None),
        ("awTp", 3, None),
        ("xsp", 2, None),
        ("xTp", 2, None),
        ("aggTp", 2, None),
        ("outTp", 2, None),
        ("outp", 2, None),
        ("ps_tp", 2, "PSUM"),
        ("ps_agg", 2, "PSUM"),
        ("ps_xt", 1, "PSUM"),
        ("ps_out", 1, "PSUM"),
    ]

    with TileContext(nc) as tc, ExitStack() as ctx:
        pools = {"f32": f32, "i32": i32, "mmdt": mmdt, "fdt": fdt}
        for name, bufs, space in pool_specs:
            kw = {"space": space} if space else {}
            pools[name] = ctx.enter_context(tc.tile_pool(name=name, bufs=bufs, **kw))

        ident = pools["consts"].tile([_P, _P], f32, tag="ident")
        nc.sync.dma_start(out=ident[:], in_=dram["ident"][:, :])
        pools["ident"] = ident
        if mm_dtype == "float32":
            pools["ident_m"] = ident
        else:
            ident_m = pools["consts"].tile([_P, _P], mmdt, tag="ident_m")
            nc.vector.tensor_copy(ident_m[:], ident[:])
            pools["ident_m"] = ident_m
        for wname, shape in (("wrel", [_C + 1, _C]), ("wroot", [_C, _C])):
            t = pools["consts"].tile(shape, f32, tag=wname)
            nc.sync.dma_start(out=t[:], in_=dram[wname][:, :])
            pools[wname + "_sb"] = t

        if square_engine == "act_sq" and mode in ("cubic", "quad"):
            hb = pools["consts"].tile([_P, 1], f32, tag="hb")
            nc.vector.memset(hb[:], float(params["h"]))
            pools["hbias_sb"] = hb

        for g in range(_G):
            _emit_graph(nc, tc, pools, g, dram, mode, params, square_engine)

    nc.finalize()
    return nc


def _get_module(w_edge, square_engine, mm_dtype, final_dtype="float32"):
    mode, params, lead = _chain_params(w_edge)
    key = (
        mode,
        tuple(sorted((k, round(v, 15)) for k, v in params.items())),
        square_engine,
        mm_dtype,
        final_dtype,
    )
    if key not in _BUILD_CACHE:
        _BUILD_CACHE[key] = _build_module(
            mode, params, square_engine, mm_dtype, final_dtype
        )
    return _BUILD_CACHE[key], lead


def _prep_inputs(x, adj, edge_attr, W_rel, b_rel, W_root, w_edge):
    x = np.ascontiguousarray(np.asarray(x, dtype=np.float32))
    adj = np.ascontiguousarray(np.asarray(adj, dtype=np.float32))
    ea = np.ascontiguousarray(np.asarray(edge_attr, dtype=np.int32).reshape(_B, _N, _N))
    W_rel = np.asarray(W_rel, dtype=np.float64)
    W_root = np.ascontiguousarray(np.asarray(W_root, dtype=np.float32))
    b_rel = np.asarray(b_rel, dtype=np.float32).reshape(1, _C)
    w_edge = np.asarray(w_edge)
    return x, adj, ea, W_rel, b_rel, W_root, w_edge


def kernel(x, adj, edge_attr, W_rel, b_rel, W_root, w_edge):
    global LAST_RESULTS
    from concourse.bass_utils import run_bass_kernel_spmd

    x, adj, ea, W_rel, b_rel, W_root, w_edge = _prep_inputs(
        x, adj, edge_attr, W_rel, b_rel, W_root, w_edge
    )
    nc, lead = _get_module(w_edge, SQUARE_ENGINE, MM_DTYPE, FINAL_DTYPE)
    wrel_eff = np.ascontiguousarray(
        np.concatenate([lead * W_rel, b_rel.astype(np.float64)], axis=0).astype(
            np.float32
        )
    )
    ident = np.eye(_P, dtype=np.float32)

    in_maps = []
    for c in range(_NCORES):
        sl = slice(c * _G, (c + 1) * _G)
        in_maps.append(
            {
                "x": x[sl],
                "adj": adj[sl],
                "ea": ea[sl],
                "wrel": wrel_eff,
                "wroot": W_root,
                "ident": ident,
            }
        )

    res = run_bass_kernel_spmd(nc, in_maps, list(range(_NCORES)), trace=TRACE)
    LAST_RESULTS = res
    out = np.concatenate([res.results[c]["out"] for c in range(_NCORES)], axis=0)
    return out


# revision 16
# speedup vs baseline: 1.1961x; 1.0500x over previous
"""DenseGATConv-style GNN message passing kernel for Trainium2 (Bass/Tile).

Math (per graph b):
    e      = w_edge[edge_attr[b]]            # [N, N] gather from 4-entry table
    adj_w  = adj[b] * e                      # weighted adjacency
    agg    = adj_w @ x[b]                    # [N, C]
    out[b] = agg @ W_rel + b_rel + x[b] @ W_root

Key tricks:
  * The 4-entry gather w_edge[a], a in {0,1,2,3}, equals the cubic polynomial
    through the 4 points, evaluated in factored form
        p(a)/c3 = (a - r) * ((a + h)^2 + v2)
    computed with one ScalarE Square activation + two fused
    scalar_tensor_tensor ops; c3 is folded into W_rel on the host.
  * b_rel is folded into the W_rel matmul as a 65th contraction row against a
    constant ones-row appended to agg^T.
  * The aggregation runs in transposed layout (out^T = Wrel^T@aggT + ...),
    with adj_w transposed on the PE in 128x128 blocks, 8 blocks batched per
    PSUM->SBUF copy.  The whole output stage runs per half-graph (512 rows)
    so the tail latency overlaps the next half's compute.

Sharding: data-parallel over batch B=16 across 8 cores (2 graphs/core);
weights replicated.
"""

import sys
from contextlib import ExitStack

sys.path.insert(0, "/opt/trn_rl_repo")

import numpy as np

_B, _N, _C = 16, 1024, 64
_NCORES = 8
_G = _B // _NCORES  # graphs per core
_P = 128
_NT = _N // _P  # 128-row tiles per graph

# Module-level knobs (test.py may flip these before calling kernel()).
TRACE = False
SQUARE_ENGINE = "act_sq"  # "act_sq" (ScalarE Square) | "dve_stt" (VectorE)
# "float32": exact, PE-bound (~4 cyc/row).  "float32r": TF32-class matmul
# precision (~1.5e-4 rel) but ~2x faster PE for transposes + aggregation.
MM_DTYPE = "float32r"
FINAL_DTYPE = "float32"  # dtype of the small output-transform matmuls
LAST_RESULTS = None  # BassKernelResults of the most recent run (for test.py)

_BUILD_CACHE = {}


def _poly_coeffs(w_edge):
    """Cubic through (k, w_edge[k]) for k=0..3, float64. Returns c0..c3."""
    w = np.asarray(w_edge, dtype=np.float64).reshape(4)
    V = np.vander(np.arange(4.0), 4, increasing=True)
    c = np.linalg.solve(V, w)
    return c  # [c0, c1, c2, c3]


def _chain_params(w_edge):
    """Pick the elementwise chain and host-folded scale from w_edge values.

    Returns (mode, params, lead) where `lead` multiplies W_rel on the host and
    the device computes adj_w/lead.
    """
    c0, c1, c2, c3 = _poly_coeffs(w_edge)
    scale = max(np.max(np.abs(np.asarray(w_edge, dtype=np.float64))), 1e-30)
    tol = 1e-7 * scale
    if abs(c3) > tol:
        # monic cubic a^3 + A a^2 + B a + C = (a - r)(a^2 + p a + q)
        A, Bc, Cc = c2 / c3, c1 / c3, c0 / c3
        roots = np.roots([1.0, A, Bc, Cc])
        r = float(np.real(roots[np.argmin(np.abs(np.imag(roots)))]))
        p = A + r
        q = Bc + p * r
        return "cubic", dict(r=r, p=p, q=q, h=p / 2.0, v2=q - p * p / 4.0), c3
    if abs(c2) > tol:
        p2, q2 = c1 / c2, c0 / c2
        return "quad", dict(p=p2, q=q2, h=p2 / 2.0, v2=q2 - p2 * p2 / 4.0), c2
    if abs(c1) > tol:
        return "linear", dict(r=-c0 / c1), c1
    return "const", dict(), c0


def _emit_square(nc, OP, AF, s_out, ea_ap, params, square_engine, pools):
    """s_out <- quadratic-part tensor; returns the constant to add to it."""
    if square_engine == "act_sq":
        nc.scalar.activation(
            s_out, ea_ap, AF.Square, bias=pools["hbias_sb"][:, 0:1], scale=1.0
        )
        return float(params["v2"])
    nc.vector.scalar_tensor_tensor(
        s_out, ea_ap, float(-params["p"]), ea_ap, OP.subtract, OP.mult
    )
    return float(params["q"])


def _emit_elementwise(
    nc, OP, AF, pools, ea_t, adj_t, mode, params, square_engine, s_pre=None
):
    """Emit adj_w/lead for one [128, N] tile slice pair; returns the aw tile."""
    sp, qtp, awp = pools["sp"], pools["qtp"], pools["awp"]
    mmdt = pools["mmdt"]
    f32 = pools["f32"]

    def square_ap():
        if s_pre is not None:
            return s_pre, float(params["v2"])
        s_t = sp.tile([_P, _N], f32)
        k = _emit_square(nc, OP, AF, s_t[:], ea_t, params, square_engine, pools)
        return s_t[:], k

    if mode == "cubic":
        qt_t = qtp.tile([_P, _N], f32)
        nc.vector.scalar_tensor_tensor(
            qt_t[:], ea_t, float(params["r"]), adj_t, OP.subtract, OP.mult
        )
        s_ap, k_add = square_ap()
        aw_t = awp.tile([_P, _N], mmdt)
        nc.vector.scalar_tensor_tensor(
            aw_t[:], s_ap, k_add, qt_t[:], OP.add, OP.mult
        )
        return aw_t
    if mode == "quad":
        s_ap, k_add = square_ap()
        aw_t = awp.tile([_P, _N], mmdt)
        nc.vector.scalar_tensor_tensor(
            aw_t[:], s_ap, k_add, adj_t, OP.add, OP.mult
        )
        return aw_t
    if mode == "linear":
        aw_t = awp.tile([_P, _N], mmdt)
        nc.vector.scalar_tensor_tensor(
            aw_t[:], ea_t, float(params["r"]), adj_t, OP.subtract, OP.mult
        )
        return aw_t
    aw_t = awp.tile([_P, _N], mmdt)
    nc.vector.tensor_copy(aw_t[:], adj_t)
    return aw_t


def _emit_half(nc, pools, g, half, dram, xs, xs_mm, xT, mode, params, square_engine):
    from concourse import mybir

    OP = mybir.AluOpType
    AF = mybir.ActivationFunctionType
    f32 = pools["f32"]
    mmdt = pools["mmdt"]
    fdt = pools["fdt"]
    adj_d, ea_d, out_d = dram["adj"], dram["ea"], dram["out"]
    ident = pools["ident"]
    ident_m = pools["ident_m"]
    H = 512

    # x^T columns for this half (root-term operand)
    p_xT = pools["ps_xt"].tile([_C, H], f32, tag="ps_xt")
    for k in range(4):
        jt = 4 * half + k
        nc.tensor.transpose(
            p_xT[:, k * _P : (k + 1) * _P],
            xs[:, jt * _C : (jt + 1) * _C],
            ident[:],
        )
    nc.scalar.copy(out=xT[:, half * H : (half + 1) * H], in_=p_xT[:])

    # DMA + elementwise for the half's 4 row-tiles (1 MiB chunks of 2 tiles)
    aw_list = []
    for pair in range(2):
        base = 4 * half + 2 * pair
        ea_t = pools["eap"].tile([_P, 2 * _N], pools["i32"])
        nc.sync.dma_start(
            out=ea_t[:].rearrange("p (q j) -> p q j", q=2),
            in_=ea_d[g, base * _P : (base + 2) * _P, :].rearrange(
                "(q p) j -> p q j", p=_P
            ),
        )
        adj_t = pools["adjp"].tile([_P, 2 * _N], f32)
        nc.sync.dma_start(
            out=adj_t[:].rearrange("p (q j) -> p q j", q=2),
            in_=adj_d[g, base * _P : (base + 2) * _P, :].rearrange(
                "(q p) j -> p q j", p=_P
            ),
        )
        s_chunk = None
        if square_engine == "act_sq" and mode in ("cubic", "quad"):
            s_chunk = pools["sp"].tile([_P, 2 * _N], f32, tag="s_chunk")
            nc.scalar.activation(
                s_chunk[:], ea_t[:], AF.Square,
                bias=pools["hbias_sb"][:, 0:1], scale=1.0,
            )
        for q in range(2):
            aw_list.append(
                _emit_elementwise(
                    nc, OP, AF, pools,
                    ea_t[:, q * _N : (q + 1) * _N],
                    adj_t[:, q * _N : (q + 1) * _N],
                    mode, params, square_engine,
                    s_pre=None if s_chunk is None
                    else s_chunk[:, q * _N : (q + 1) * _N],
                )
            )

    # transpose adj_w blocks (jt-pair batched) + accumulate agg^T over j
    p_aggT = pools["ps_agg"].tile([_C, H], f32, tag="ps_agg")
    for jtp in range(4):
        p_tp = pools["ps_tp"].tile([_P, 2 * H], mmdt, tag="ps_tp")
        for sub in range(2):
            jt = 2 * jtp + sub
            for k in range(4):
                nc.tensor.transpose(
                    p_tp[:, sub * H + k * _P : sub * H + (k + 1) * _P],
                    aw_list[k][:, jt * _P : (jt + 1) * _P],
                    ident_m[:],
                )
        awT = pools["awTp"].tile([_P, 2 * H], mmdt)
        nc.scalar.copy(out=awT[:], in_=p_tp[:])
        for sub in range(2):
            jt = 2 * jtp + sub
            nc.tensor.matmul(
                p_aggT[:],
                lhsT=xs_mm[:, jt * _C : (jt + 1) * _C],
                rhs=awT[:, sub * H : (sub + 1) * H],
                start=(jt == 0),
                stop=(jt == _NT - 1),
            )

    aggT = pools["aggTp"].tile([_C + 1, H], fdt)
    nc.vector.memset(aggT[_C : _C + 1, :], 1.0)
    nc.scalar.copy(out=aggT[:_C, :], in_=p_aggT[:])

    # out^T[c', i-half] = [W_rel; b_rel]^T @ [aggT; 1] + W_root^T @ xT
    p_out = pools["ps_out"].tile([_C, H], f32, tag="ps_out")
    nc.tensor.matmul(
        p_out[:], lhsT=pools["wrel_sb"][:], rhs=aggT[:], start=True, stop=False
    )
    nc.tensor.matmul(
        p_out[:], lhsT=pools["wroot_sb"][:],
        rhs=xT[:, half * H : (half + 1) * H] if fdt is f32
        else pools["xT_mm"][:, half * H : (half + 1) * H],
        start=False, stop=True,
    )
    outT = pools["outTp"].tile([_C, H], f32)
    nc.scalar.copy(out=outT[:], in_=p_out[:])

    # back to natural [i, c] layout and store
    p_on = pools["ps_out"].tile([_P, 4 * _C], f32, tag="ps_out")
    for k in range(4):
        nc.tensor.transpose(
            p_on[:, k * _C : (k + 1) * _C],
            outT[:, k * _P : (k + 1) * _P],
            ident[:_C, :_C],
        )
    out_sb = pools["outp"].tile([_P, 4 * _C], f32)
    nc.scalar.copy(out=out_sb[:], in_=p_on[:])
    nc.sync.dma_start(
        out=out_d[g, half * H : (half + 1) * H, :].rearrange(
            "(t p) c -> p t c", p=_P
        ),
        in_=out_sb[:].rearrange("p (t c) -> p t c", t=4),
    )


def _emit_graph(nc, tc, pools, g, dram, mode, params, square_engine):
    f32 = pools["f32"]
    mmdt = pools["mmdt"]
    x_d = dram["x"]

    # x in aggregation layout: xs[p, t*C+c] = x[t*128+p, c]
    xs = pools["xsp"].tile([_P, _NT * _C], f32)
    nc.sync.dma_start(
        out=xs[:].rearrange("p (t c) -> p t c", t=_NT),
        in_=x_d[g, :, :].rearrange("(t p) c -> p t c", p=_P),
    )
    if mmdt is f32:
        xs_mm = xs
    else:
        xs_mm = pools["xsp"].tile([_P, _NT * _C], mmdt, tag="xs_mm")
        nc.vector.tensor_copy(xs_mm[:], xs[:])
    xT = pools["xTp"].tile([_C, _N], f32)

    for half in range(2):
        _emit_half(
            nc, pools, g, half, dram, xs, xs_mm, xT, mode, params, square_engine
        )


def _build_module(mode, params, square_engine, mm_dtype, final_dtype):
    import concourse.bass as bass  # noqa: F401
    from concourse import bacc, mybir
    from concourse.tile import TileContext

    f32 = mybir.dt.float32
    i32 = mybir.dt.int32
    mmdt = getattr(mybir.dt, mm_dtype)
    fdt = getattr(mybir.dt, final_dtype)
    assert fdt is f32, "FINAL_DTYPE other than float32 not wired up"

    nc = bacc.Bacc(
        "TRN2", target_bir_lowering=False, debug=False, num_devices=_NCORES
    )

    dram = {
        "x": nc.dram_tensor("x", [_G, _N, _C], f32, kind="ExternalInput"),
        "adj": nc.dram_tensor("adj", [_G, _N, _N], f32, kind="ExternalInput"),
        "ea": nc.dram_tensor("ea", [_G, _N, _N], i32, kind="ExternalInput"),
        "wrel": nc.dram_tensor("wrel", [_C + 1, _C], f32, kind="ExternalInput"),
        "wroot": nc.dram_tensor("wroot", [_C, _C], f32, kind="ExternalInput"),
        "ident": nc.dram_tensor("ident", [_P, _P], f32, kind="ExternalInput"),
        "out": nc.dram_tensor("out", [_G, _N, _C], f32, kind="ExternalOutput"),
    }

    pool_specs = [
        ("consts", 1, None),
        ("adjp", 4, None),
        ("eap", 4, None),
        ("sp", 3, None),
        ("qtp", 3, None),
        ("awp", 6, None),
        ("awTp", 3, None),
        ("xsp", 2, None),
        ("xTp", 2, None),
        ("aggTp", 2, None),
        ("outTp", 2, None),
        ("outp", 2, None),
        ("ps_tp", 2, "PSUM"),
        ("ps_agg", 2, "PSUM"),
        ("ps_xt", 1, "PSUM"),
        ("ps_out", 1, "PSUM"),
    ]

    with TileContext(nc) as tc, ExitStack() as ctx:
        pools = {"f32": f32, "i32": i32, "mmdt": mmdt, "fdt": fdt}
        for name, bufs, space in pool_specs:
            kw = {"space": space} if space else {}
            pools[name] = ctx.enter_context(tc.tile_pool(name=name, bufs=bufs, **kw))

        ident = pools["consts"].tile([_P, _P], f32, tag="ident")
        nc.sync.dma_start(out=ident[:], in_=dram["ident"][:, :])
        pools["ident"] = ident
        if mm_dtype == "float32":
            pools["ident_m"] = ident
        else:
            ident_m = pools["consts"].tile([_P, _P], mmdt, tag="ident_m")
            nc.vector.tensor_copy(ident_m[:], ident[:])
            pools["ident_m"] = ident_m
        for wname, shape in (("wrel", [_C + 1, _C]), ("wroot", [_C, _C])):
            t = pools["consts"].tile(shape, f32, tag=wname)
            nc.sync.dma_start(out=t[:], in_=dram[wname][:, :])
            pools[wname + "_sb"] = t

        if square_engine == "act_sq" and mode in ("cubic", "quad"):
            hb = pools["consts"].tile([_P, 1], f32, tag="hb")
            nc.vector.memset(hb[:], float(params["h"]))
            pools["hbias_sb"] = hb

        for g in range(_G):
            _emit_graph(nc, tc, pools, g, dram, mode, params, square_engine)

    nc.finalize()
    return nc


def _get_module(w_edge, square_engine, mm_dtype, final_dtype="float32"):
    mode, params, lead = _chain_params(w_edge)
    key = (
        mode,
        tuple(sorted((k, round(v, 15)) for k, v in params.items())),
        square_engine,
        mm_dtype,
        final_dtype,
    )
    if key not in _BUILD_CACHE:
        _BUILD_CACHE[key] = _build_module(
            mode, params, square_engine, mm_dtype, final_dtype
        )
    return _BUILD_CACHE[key], lead


def _prep_inputs(x, adj, edge_attr, W_rel, b_rel, W_root, w_edge):
    x = np.ascontiguousarray(np.asarray(x, dtype=np.float32))
    adj = np.ascontiguousarray(np.asarray(adj, dtype=np.float32))
    ea = np.ascontiguousarray(np.asarray(edge_attr, dtype=np.int32).reshape(_B, _N, _N))
    W_rel = np.asarray(W_rel, dtype=np.float64)
    W_root = np.ascontiguousarray(np.asarray(W_root, dtype=np.float32))
    b_rel = np.asarray(b_rel, dtype=np.float32).reshape(1, _C)
    w_edge = np.asarray(w_edge)
    return x, adj, ea, W_rel, b_rel, W_root, w_edge


def kernel(x, adj, edge_attr, W_rel, b_rel, W_root, w_edge):
    global LAST_RESULTS
    from concourse.bass_utils import run_bass_kernel_spmd

    x, adj, ea, W_rel, b_rel, W_root, w_edge = _prep_inputs(
        x, adj, edge_attr, W_rel, b_rel, W_root, w_edge
    )
    nc, lead = _get_module(w_edge, SQUARE_ENGINE, MM_DTYPE, FINAL_DTYPE)
    wrel_eff = np.ascontiguousarray(
        np.concatenate([lead * W_rel, b_rel.astype(np.float64)], axis=0).astype(
            np.float32
        )
    )
    ident = np.eye(_P, dtype=np.float32)

    in_maps = []
    for c in range(_NCORES):
        sl = slice(c * _G, (c + 1) * _G)
        in_maps.append(
            {
                "x": x[sl],
                "adj": adj[sl],
                "ea": ea[sl],
                "wrel": wrel_eff,
                "wroot": W_root,
                "ident": ident,
            }
        )

    res = run_bass_kernel_spmd(nc, in_maps, list(range(_NCORES)), trace=TRACE)
    LAST_RESULTS = res
    out = np.concatenate([res.results[c]["out"] for c in range(_NCORES)], axis=0)
    return out
